# revision 52
# baseline (speedup 1.0000x reference)
"""MobileMQA Trainium2 kernel (8 NeuronCores, SPMD).

Reference computation (per batch b of 2):
  q  = x @ wq + bq                         [1024 tok, 512]
  kv = x @ wkv + bkv                       [1024 tok, 1024]
  kv = depthwise3x3_s2_same(kv) + dw_bias  [256 sp, 1024]
  k, v = split(kv)  -> reshape to shared-KV length M=2048 (channel fold)
  attn = softmax(q @ k^T * 0.125); out = attn @ v
  y = out @ wo + bo

Sharding: core c handles batch b=c//4, query chunk j=c%4 (256 tokens).
KV path (proj+conv) is replicated across the 4 cores of a batch (MQA).

v2 design: single software-pipelined emission stream.  Attention m-groups
(4 m-tiles each) interleave into the kv/conv phase as soon as their k/v
ch-tile pair is ready (softmax is m-order invariant, so m is re-grouped by
channel-fold group instead of spatial order).  Heads are split in two
halves (heads 0-3 / 4-7) so the AV accumulator fits PSUM:
  avt [128, 1024] f32 (2 banks, ring of 1, rows 64:127 = softmax denom
  via ones-columns in the V tiles) + st [128,1024] x2 (4 banks) +
  kvp [128,512] (1) + cvp [128,256] (1) = 8 banks.
Half 0 runs inside the kv phase; half 1 + output proj form the tail,
with wo-contraction partials issued per completed attnT tile.
V tiles are built by SBUF->SBUF DMA transpose (bf16); k/q stay f32r.
"""
import sys

for _p in ("/opt/trn_rl_repo", "/opt/trn_rl_repo/concourse"):
    if _p not in sys.path:
        sys.path.insert(0, _p)

import numpy as np

import concourse.bass as bass
import concourse.mybir as mybir
import concourse.tile as tile
from concourse import bacc
from concourse.bass_utils import run_bass_kernel_spmd
from concourse.masks import make_identity

F32 = mybir.dt.float32
F32R = mybir.dt.float32r
BF16 = mybir.dt.bfloat16
AF = mybir.ActivationFunctionType
ALU = mybir.AluOpType

DIM = 512
NH = 8
HD = 64
B, H, W = 2, 32, 32
L = H * W            # 1024 tokens per batch
KH = KW = 16
NS = KH * KW         # 256 conv-output spatial positions
M = NS * NH          # 2048 shared-KV positions
CH = 2 * DIM         # 1024 kv channels
SCALE = HD ** -0.5   # 0.125
PADW = 33            # padded conv input row (32 + 1 SAME-pad)
NPAD = PADW * PADW   # 1089

_NC_CACHE = {}


def _round_f32r(a: np.ndarray) -> np.ndarray:
    """Round fp32 to the fp32r grid (11-bit mantissa, round-to-nearest)."""
    bits = np.ascontiguousarray(a, np.float32).view(np.uint32)
    bits = (bits + np.uint32(0x800)) & np.uint32(0xFFFFF000)
    return bits.view(np.float32)


def _build_program():
    nc = bacc.Bacc(None)

    xT_d = nc.dram_tensor("xT", [DIM, L], BF16, kind="ExternalInput")
    xTc_d = nc.dram_tensor("xTc", [DIM, 256], BF16, kind="ExternalInput")
    wkv_d = nc.dram_tensor("wkv", [DIM, CH], BF16, kind="ExternalInput")
    wq_d = nc.dram_tensor("wq", [DIM, DIM], BF16, kind="ExternalInput")
    wo_d = nc.dram_tensor("wo", [DIM, DIM], F32R, kind="ExternalInput")
    bpl_d = nc.dram_tensor("bpl", [CH, NS], BF16, kind="ExternalInput")
    # consts: cols 0-3 bq tiles, 4-75 dw weights (8 ch-tiles x 9 taps), 76-79 bo
    cst_d = nc.dram_tensor("cst", [128, 80], F32, kind="ExternalInput")
    y_d = nc.dram_tensor("y", [DIM, 256], F32, kind="ExternalOutput")

    xT_r = xT_d[:, :].rearrange("(k p) t -> p k t", p=128)
    wkv_r = wkv_d[:, :].rearrange("(k p) c -> p k c", p=128)
    bpl_r = bpl_d[:, :].rearrange("(t p) s -> p t s", p=128)

    with tile.TileContext(nc) as tc:
        with tc.tile_pool(name="wp", bufs=1) as wp, \
             tc.tile_pool(name="expp", bufs=6) as expp, \
             tc.tile_pool(name="caccp", bufs=3) as caccp, \
             tc.tile_pool(name="zrp", bufs=2) as zrp, \
             tc.tile_pool(name="stp", bufs=2, space="PSUM") as stp, \
             tc.tile_pool(name="avp", bufs=1, space="PSUM") as avp:

            # ---------------- persistent SBUF + input DMAs ----------------
            cst = wp.tile([128, 80], F32, tag="cst")
            xT = wp.tile([128, 4, L], BF16, tag="xT")
            wkv = wp.tile([128, 4, CH], BF16, tag="wkv")
            wq = wp.tile([128, 4, DIM], BF16, tag="wq")
            wo = wp.tile([128, 4, DIM], F32R, tag="wo")
            xTc = wp.tile([128, 4, 256], BF16, tag="xTc")
            bpl = wp.tile([128, 8, NS], BF16, tag="bpl")

            # DMA issue order == arrival order (single transfer channel).
            # Host permutes wkv cols / bpl tiles into consumption-pair order
            # [k0|v0|k1|v1|...] so each group's weights land as one
            # descriptor (HWDGE descriptor time dominates small DMAs).
            def load_wkv_grp(t):
                nc.sync.dma_start(out=wkv[:, :, t * 256:(t + 1) * 256],
                                  in_=wkv_r[:, :, t * 256:(t + 1) * 256])

            # first inputs dual-queued (SP + ACT hwdge) for descriptor
            # parallelism at startup
            nc.scalar.dma_start(out=wkv[:, :, 0:256], in_=wkv_r[:, :, 0:256])
            nc.scalar.dma_start(out=wq,
                                in_=wq_d[:, :].rearrange("(k p) c -> p k c", p=128))
            nc.scalar.dma_start(out=cst, in_=cst_d[:, :])
            nc.sync.dma_start(out=xT[:, :, 0:512], in_=xT_r[:, :, 0:512])
            nc.sync.dma_start(out=xTc,
                              in_=xTc_d[:, :].rearrange("(k p) t -> p k t", p=128))
            nc.sync.dma_start(out=bpl[:, 0:2, :], in_=bpl_r[:, 0:2, :])
            nc.sync.dma_start(out=xT[:, :, 512:L], in_=xT_r[:, :, 512:L])
            load_wkv_grp(1)
            nc.sync.dma_start(out=bpl[:, 2:4, :], in_=bpl_r[:, 2:4, :])
            load_wkv_grp(2)
            nc.sync.dma_start(out=bpl[:, 4:8, :], in_=bpl_r[:, 4:8, :])
            load_wkv_grp(3)
            nc.sync.dma_start(out=wo,
                              in_=wo_d[:, :].rearrange("(k p) c -> p k c", p=128))

            # ---------------- setup (runs during DMA window) ----------------
            ident = wp.tile([128, 128], F32, tag="ident")
            make_identity(nc, ident)
            warm = wp.tile([1, 1], F32, tag="warm")
            nc.vector.memset(warm, 0.0)
            nc.scalar.activation(warm[:, :], warm[:, :], AF.Exp)

            # conv input planes: two pinned buffers, zero pad written once
            # (right pad col 32, bottom pad row 32; interior rewritten per tile)
            kvsb = [wp.tile([128, NPAD], BF16, tag=f"kvsb{i}", name=f"kvsb{i}")
                    for i in range(2)]
            zpad = wp.tile([128, PADW], F32, tag="zpad")
            nc.gpsimd.memset(zpad, 0.0)
            for i in range(2):
                pad_col = bass.AP(tensor=kvsb[i].tensor,
                                  offset=kvsb[i].offset + 32,
                                  ap=[kvsb[i].ap[0], [PADW, PADW]])
                nc.gpsimd.tensor_copy(pad_col, zpad[:, :])
                nc.gpsimd.tensor_copy(kvsb[i][:, PADW * 32: PADW * 32 + 32],
                                      zpad[:, 0:32])

            # V tiles [128 m, 128]: cols 0:64 = V^T chunk, cols 64:128 = ones
            # (AV matmul then yields softmax denominator on out partitions
            # 64:127 for free).
            vaug = wp.tile([128, 16, 128], BF16, tag="vaug")
            ones_f = wp.tile([128, 1024], F32, tag="ones_f")
            nc.gpsimd.memset(ones_f, 1.0)
            ones_dst = bass.AP(tensor=vaug.tensor, offset=vaug.offset + 64,
                               ap=[vaug.ap[0], [128, 16], [1, 64]])
            nc.gpsimd.tensor_copy(ones_dst,
                                  ones_f[:, :].rearrange("p (a b) -> p a b", b=64))

            kT2 = wp.tile([128, M], F32R, tag="kT2")
            qT2 = wp.tile([128, NH, 256], F32R, tag="qT2")
            attnT = [wp.tile([128, 256], F32R, tag=f"attnT{i}", name=f"attnT{i}")
                     for i in range(4)]
            ysb = wp.tile([128, 4, 256], F32, tag="ysb")

            # conv diagonal weights (Pool engine, from cst).  Emitted lazily,
            # one group ahead of use, so Pool's in-order queue stays
            # responsive for the kT2 copies the attention steps wait on.
            diags = {}

            def emit_diags(t):
                for t_i in (t, 4 + t):
                    for tap in range(6):
                        d = wp.tile([128, 128], BF16, tag=f"dg{t_i}_{tap}",
                                    name=f"dg{t_i}_{tap}")
                        nc.gpsimd.tensor_scalar_mul(
                            d[:, :], ident[:, :],
                            cst[:, 4 + 9 * t_i + tap: 5 + 9 * t_i + tap])
                        diags[(t_i, tap)] = d

            emit_diags(0)
            emit_diags(1)

            with tc.tile_pool(name="kvps", bufs=1, space="PSUM") as kvps, \
                 tc.tile_pool(name="cvps", bufs=1, space="PSUM") as cvps:

                # ---------------- pipeline building blocks ----------------
                def kv_chunk(t_i, buf, n, pad_dve):
                    """Project one 512-token chunk of ch-tile t_i into the
                    padded conv plane (rows 16n..16n+16).  The pad copy
                    alternates ACT/DVE to balance engine load."""
                    kvp = kvps.tile([128, 512], F32, tag="kvp")
                    wc = (2 * t_i if t_i < 4 else 2 * (t_i - 4) + 1) * 128
                    for k in range(4):
                        nc.tensor.matmul(kvp[:, :],
                                         wkv[:, k, wc:wc + 128],
                                         xT[:, k, n * 512:(n + 1) * 512],
                                         start=(k == 0), stop=(k == 3))
                    dst = bass.AP(tensor=buf.tensor,
                                  offset=buf.offset + PADW * 16 * n,
                                  ap=[buf.ap[0], [PADW, 16], [1, 32]])
                    src = kvp[:, :].rearrange("p (a b) -> p a b", b=32)
                    if pad_dve:
                        nc.vector.tensor_copy(dst, src)
                    else:
                        nc.scalar.copy(dst, src)

                def conv(t_i, buf, out_dtype, fuse=False):
                    # taps 0-5 on PE (diag matmuls); taps 6-8 as a DVE
                    # stt chain on top (saves 2.6us of PE in the PE-bound
                    # part A; DVE has slack).
                    cvp = cvps.tile([128, NS], F32, tag="cvp")
                    for tap in range(6):
                        dy, dx = tap // 3, tap % 3
                        win = bass.AP(tensor=buf.tensor,
                                      offset=buf.offset + PADW * dy + dx,
                                      ap=[buf.ap[0], [2 * PADW, KH], [2, KW]])
                        nc.tensor.matmul(cvp[:, :], diags[(t_i, tap)][:, :], win,
                                         start=(tap == 0), stop=(tap == 5))
                    # DVE accumulates taps 6-8 (seeded with the bias plane)
                    # in parallel with the PE taps; one combine at the end.
                    cacc = caccp.tile([128, NS], out_dtype, tag=f"cacc{out_dtype}")
                    bi = 2 * t_i if t_i < 4 else 2 * (t_i - 4) + 1
                    for tap in range(6, 9):
                        dy, dx = tap // 3, tap % 3
                        win = bass.AP(tensor=buf.tensor,
                                      offset=buf.offset + PADW * dy + dx,
                                      ap=[buf.ap[0], [2 * PADW, KH], [2, KW]])
                        nc.vector.scalar_tensor_tensor(
                            cacc[:, :], win,
                            cst[:, 4 + 9 * t_i + tap: 5 + 9 * t_i + tap],
                            bpl[:, bi, :] if tap == 6 else cacc[:, :],
                            op0=ALU.mult, op1=ALU.add)
                    if fuse:
                        return cvp, cacc
                    nc.vector.scalar_tensor_tensor(
                        cacc[:, :], cvp[:, :], 1.0, cacc[:, :],
                        op0=ALU.mult, op1=ALU.add)
                    return cacc

                def k_assemble(t, cvp, cacc):
                    # fused: kT2 = cvp (PE taps) + cacc (DVE taps + bias)
                    for gi in range(2):
                        mt0 = 4 * t + 2 * gi
                        nc.vector.scalar_tensor_tensor(
                            kT2[0:64, mt0 * 128:(mt0 + 2) * 128],
                            cvp[gi * 64:gi * 64 + 64, :], 1.0,
                            cacc[gi * 64:gi * 64 + 64, :],
                            op0=ALU.mult, op1=ALU.add)

                def v_assemble(t, cacc):
                    # PE transpose into an st-ring PSUM tile (no extra bank),
                    # then DVE copy (f32 -> bf16) into the vaug layout.
                    for gi in range(2):
                        vt = stp.tile([128, 1024], F32, tag="st")
                        for sh in range(2):
                            nc.tensor.transpose(
                                vt[:, sh * 512:sh * 512 + 64],
                                cacc[gi * 64:gi * 64 + 64,
                                     sh * 128:(sh + 1) * 128],
                                ident[gi * 64:gi * 64 + 64,
                                      gi * 64:gi * 64 + 64])
                        for sh in range(2):
                            mt = 4 * t + 2 * gi + sh
                            nc.vector.tensor_copy(vaug[:, mt, 0:64],
                                                  vt[:, sh * 512:sh * 512 + 64])

                def q_half(half):
                    qp = stp.tile([128, 1024], F32, tag="st")
                    for ti in range(2):
                        t_i = 2 * half + ti
                        for k in range(4):
                            nc.tensor.matmul(
                                qp[:, ti * 256:(ti + 1) * 256],
                                wq[:, k, t_i * 128:(t_i + 1) * 128],
                                xTc[:, k, :],
                                start=(k == 0), stop=(k == 3))
                    for ti in range(2):
                        t_i = 2 * half + ti
                        for gi in range(2):          # head 2*t_i + gi
                            h = 2 * t_i + gi
                            nc.vector.tensor_scalar_add(
                                qT2[0:64, h, :],
                                qp[gi * 64:gi * 64 + 64,
                                   ti * 256:(ti + 1) * 256],
                                cst[gi * 64:gi * 64 + 64, t_i:t_i + 1])

                def scores_step(mt, half):
                    """Scores + exp for one m-tile / head-half; returns ex."""
                    st = stp.tile([128, 1024], F32, tag="st")
                    for n in range(2):
                        nc.tensor.matmul(
                            st[:, n * 512:(n + 1) * 512],
                            kT2[0:64, mt * 128:(mt + 1) * 128],
                            qT2[0:64, 4 * half + 2 * n:4 * half + 2 * n + 2, :],
                            start=True, stop=True)
                    ex = expp.tile([128, 1024], BF16, tag="ex")
                    nc.scalar.activation(ex[:, :], st[:, :], AF.Exp,
                                         scale=float(SCALE))
                    return ex

                def av_step(mt, avt, ex):
                    for n in range(2):
                        nc.tensor.matmul(
                            avt[:, n * 512:(n + 1) * 512],
                            vaug[:, mt, :],
                            ex[:, n * 512:(n + 1) * 512],
                            start=(mt == 0), stop=(mt == 15))

                def normalize(half, avt):
                    """avt -> attnT[2*half], attnT[2*half+1] (cols = tokens).
                    Reciprocal split in 256-col chunks to shorten the DVE
                    serial chain at the tail."""
                    zr = zrp.tile([128, 1024], F32, tag="zr")
                    nc.vector.reciprocal(zr[0:64, :], avt[64:128, :])
                    for hh in range(4):              # head 4*half + hh
                        h = 4 * half + hh
                        nc.vector.scalar_tensor_tensor(
                            attnT[h // 2][(h % 2) * 64:(h % 2) * 64 + 64, :],
                            avt[0:64, hh * 256:(hh + 1) * 256], 1.0,
                            zr[0:64, hh * 256:(hh + 1) * 256],
                            op0=ALU.mult, op1=ALU.mult)

                # ---------------- part A: kv/conv/q + head-half 0 ----------------
                # Fine-grained weave: the exp latency of an attention step and
                # the pad-copy latency of a kv chunk are covered by emitting
                # the next block of independent PE work between the dependent
                # pairs (PE executes in emission order).
                avt0 = avp.tile([128, 1024], F32, tag="avt")

                def group(t):
                    f = [4 * (t - 1) + i for i in range(4)]   # fill steps
                    kv_chunk(t, kvsb[0], 0, pad_dve=False)
                    kv_chunk(t, kvsb[0], 1, pad_dve=False)
                    ex0 = scores_step(f[0], 0) if t > 0 else None
                    if t == 0:
                        q_half(0)
                    cvp_k, cacc_k = conv(t, kvsb[0], F32R, fuse=True)
                    if t > 0:
                        av_step(f[0], avt0, ex0)
                        ex1 = scores_step(f[1], 0)
                    k_assemble(t, cvp_k, cacc_k)
                    kv_chunk(4 + t, kvsb[1], 0, pad_dve=False)
                    kv_chunk(4 + t, kvsb[1], 1, pad_dve=False)
                    if t > 0:
                        av_step(f[1], avt0, ex1)
                        ex2 = scores_step(f[2], 0)
                    else:
                        q_half(1)
                    cacc_v = conv(4 + t, kvsb[1], F32)
                    if t > 0:
                        av_step(f[2], avt0, ex2)
                        ex3 = scores_step(f[3], 0)
                    v_assemble(t, cacc_v)
                    if t > 0:
                        av_step(f[3], avt0, ex3)
                    if t < 2:
                        emit_diags(t + 2)

                for t in range(4):
                    group(t)
                # drain the last m-group's half-0 steps (software pipelined)
                exq = scores_step(12, 0)
                for mt in range(13, 16):
                    exn = scores_step(mt, 0)
                    av_step(mt - 1, avt0, exq)
                    exq = exn
                av_step(15, avt0, exq)
                normalize(0, avt0)

            # ---------------- part B: head-half 1 + output proj ----------------
            # yp tiles pack two m-tiles per PSUM bank so all four output
            # accumulators are live; the wo-contraction partials for k=0,1
            # (attnT[0]/[1], ready since half 0) fill PE idle during the
            # ACT-paced half-1 attention steps.
            with tc.tile_pool(name="yps", bufs=1, space="PSUM") as yps:
                avt1 = avp.tile([128, 1024], F32, tag="avt")

                def yproj_pair(m, k0):
                    """wo-contraction partial (k0, k0+1) for m-tile m; a
                    closed PSUM group, accumulated into ysb by DVE.  Two
                    alternating tags so DVE accumulation never blocks the
                    next pair's matmuls."""
                    ypt = yps.tile([128, 256], F32, tag=f"yp{m % 2}",
                                   name=f"yp_{m}_{k0}")
                    for k in (k0, k0 + 1):
                        nc.tensor.matmul(ypt[:, :],
                                         wo[:, k, m * 128:(m + 1) * 128],
                                         attnT[k][:, :],
                                         start=(k == k0), stop=(k == k0 + 1))
                    if k0 == 0:
                        nc.vector.tensor_scalar_add(ysb[:, m, :], ypt[:, :],
                                                    cst[:, 76 + m:77 + m])
                    else:
                        nc.vector.scalar_tensor_tensor(
                            ysb[:, m, :], ypt[:, :], 1.0, ysb[:, m, :],
                            op0=ALU.mult, op1=ALU.add)

                exq = scores_step(0, 1)
                for mt in range(1, 16):
                    exn = scores_step(mt, 1)
                    av_step(mt - 1, avt1, exq)
                    exq = exn
                    if 1 <= mt < 5:
                        yproj_pair(mt - 1, 0)   # k=0,1 from half-0 heads
                av_step(15, avt1, exq)
                normalize(1, avt1)
                for m in range(4):
                    yproj_pair(m, 2)            # k=2,3 from half-1 heads
                    eng = nc.sync if m % 2 == 0 else nc.scalar
                    eng.dma_start(
                        out=y_d[m * 128:(m + 1) * 128, :],
                        in_=ysb[:, m, :])

    nc.finalize()
    return nc


def _get_program():
    if "nc" not in _NC_CACHE:
        _NC_CACHE["nc"] = _build_program()
    return _NC_CACHE["nc"]


def _host_prep(x, wq, bq, wkv, bkv, dw_kernel, dw_bias, wo, bo):
    """Build the 8 per-core input maps."""
    import ml_dtypes
    BF = ml_dtypes.bfloat16
    x = np.ascontiguousarray(np.asarray(x, np.float32))
    wq_r = np.asarray(wq, np.float32).astype(BF)
    wkv_p = np.asarray(wkv, np.float32).reshape(DIM, 8, 128)
    perm = [0, 4, 1, 5, 2, 6, 3, 7]          # [k0|v0|k1|v1|...]
    wkv_r = np.ascontiguousarray(wkv_p[:, perm, :].reshape(DIM, CH)).astype(BF)
    wo_r = _round_f32r(np.asarray(wo, np.float32))
    bq = np.asarray(bq, np.float32)
    bkv = np.asarray(bkv, np.float32)
    dw_bias = np.asarray(dw_bias, np.float32)
    bo = np.asarray(bo, np.float32)
    dww = np.asarray(dw_kernel, np.float32).reshape(9, CH).T.copy()  # [1024, 9]

    # bias plane: dw_bias + bkv * sum(valid taps), SAME padding aware
    oy = np.arange(KH)
    valid_y = (2 * oy[:, None] + np.arange(3)[None, :]) < H      # [16, 3]
    valid_x = valid_y.copy()
    wsum = np.zeros((CH, KH, KW), np.float32)
    for tap in range(9):
        dy, dx = tap // 3, tap % 3
        m2 = np.outer(valid_y[:, dy], valid_x[:, dx]).astype(np.float32)
        wsum += dww[:, tap][:, None, None] * m2[None, :, :]
    bpl = (dw_bias[:, None] + bkv[:, None] * wsum.reshape(CH, NS))
    bpl = np.ascontiguousarray(
        bpl.reshape(8, 128, NS)[[0, 4, 1, 5, 2, 6, 3, 7]].reshape(CH, NS)).astype(BF)

    cst = np.zeros((128, 80), np.float32)
    cst[:, 0:4] = bq.reshape(4, 128).T
    for t_i in range(8):
        cst[:, 4 + 9 * t_i: 13 + 9 * t_i] = dww[t_i * 128:(t_i + 1) * 128, :]
    cst[:, 76:80] = bo.reshape(4, 128).T

    in_maps = []
    for c in range(8):
        b, j = c // 4, c % 4
        xT = x[b].reshape(L, DIM).T.astype(BF)
        in_maps.append({
            "xT": np.ascontiguousarray(xT),
            "xTc": np.ascontiguousarray(xT[:, j * 256:(j + 1) * 256]),
            "wkv": wkv_r, "wq": wq_r, "wo": wo_r,
            "bpl": bpl, "cst": cst,
        })
    return in_maps


def kernel(**inputs) -> np.ndarray:
    nc = _get_program()
    in_maps = _host_prep(**inputs)
    res = run_bass_kernel_spmd(nc, in_maps, core_ids=list(range(8)))
    out = np.zeros((B, H, W, DIM), np.float32)
    flat = out.reshape(B, L, DIM)
    for c in range(8):
        b, j = c // 4, c % 4
        flat[b, j * 256:(j + 1) * 256, :] = res.results[c]["y"].T
    return out


# revision 53
# speedup vs baseline: 1.0209x; 1.0209x over previous
"""MobileMQA Trainium2 kernel (8 NeuronCores, SPMD).

Reference computation (per batch b of 2):
  q  = x @ wq + bq                         [1024 tok, 512]
  kv = x @ wkv + bkv                       [1024 tok, 1024]
  kv = depthwise3x3_s2_same(kv) + dw_bias  [256 sp, 1024]
  k, v = split(kv)  -> reshape to shared-KV length M=2048 (channel fold)
  attn = softmax(q @ k^T * 0.125); out = attn @ v
  y = out @ wo + bo

Sharding: core c handles batch b=c//4, query chunk j=c%4 (256 tokens).
KV path (proj+conv) is replicated across the 4 cores of a batch (MQA).

v2 design: single software-pipelined emission stream.  Attention m-groups
(4 m-tiles each) interleave into the kv/conv phase as soon as their k/v
ch-tile pair is ready (softmax is m-order invariant, so m is re-grouped by
channel-fold group instead of spatial order).  Heads are split in two
halves (heads 0-3 / 4-7) so the AV accumulator fits PSUM:
  avt [128, 1024] f32 (2 banks, ring of 1, rows 64:127 = softmax denom
  via ones-columns in the V tiles) + st [128,1024] x2 (4 banks) +
  kvp [128,512] (1) + cvp [128,256] (1) = 8 banks.
Half 0 runs inside the kv phase; half 1 + output proj form the tail,
with wo-contraction partials issued per completed attnT tile.
V tiles are built by SBUF->SBUF DMA transpose (bf16); k/q stay f32r.
"""
import sys

for _p in ("/opt/trn_rl_repo", "/opt/trn_rl_repo/concourse"):
    if _p not in sys.path:
        sys.path.insert(0, _p)

import numpy as np

import concourse.bass as bass
import concourse.mybir as mybir
import concourse.tile as tile
from concourse import bacc
from concourse.bass_utils import run_bass_kernel_spmd
from concourse.masks import make_identity

F32 = mybir.dt.float32
F32R = mybir.dt.float32r
BF16 = mybir.dt.bfloat16
AF = mybir.ActivationFunctionType
ALU = mybir.AluOpType

DIM = 512
NH = 8
HD = 64
B, H, W = 2, 32, 32
L = H * W            # 1024 tokens per batch
KH = KW = 16
NS = KH * KW         # 256 conv-output spatial positions
M = NS * NH          # 2048 shared-KV positions
CH = 2 * DIM         # 1024 kv channels
SCALE = HD ** -0.5   # 0.125
PADW = 33            # padded conv input row (32 + 1 SAME-pad)
NPAD = PADW * PADW   # 1089

_NC_CACHE = {}


def _round_f32r(a: np.ndarray) -> np.ndarray:
    """Round fp32 to the fp32r grid (11-bit mantissa, round-to-nearest)."""
    bits = np.ascontiguousarray(a, np.float32).view(np.uint32)
    bits = (bits + np.uint32(0x800)) & np.uint32(0xFFFFF000)
    return bits.view(np.float32)


def _build_program():
    nc = bacc.Bacc(None)

    xT_d = nc.dram_tensor("xT", [DIM, L], BF16, kind="ExternalInput")
    xTc_d = nc.dram_tensor("xTc", [DIM, 256], BF16, kind="ExternalInput")
    wkv_d = nc.dram_tensor("wkv", [DIM, CH], BF16, kind="ExternalInput")
    wq_d = nc.dram_tensor("wq", [DIM, DIM], BF16, kind="ExternalInput")
    wo_d = nc.dram_tensor("wo", [DIM, DIM], F32R, kind="ExternalInput")
    bpl_d = nc.dram_tensor("bpl", [CH, NS], BF16, kind="ExternalInput")
    # consts: cols 0-3 bq tiles, 4-75 dw weights (8 ch-tiles x 9 taps), 76-79 bo
    cst_d = nc.dram_tensor("cst", [128, 80], F32, kind="ExternalInput")
    y_d = nc.dram_tensor("y", [DIM, 256], F32, kind="ExternalOutput")

    xT_r = xT_d[:, :].rearrange("(k p) t -> p k t", p=128)
    wkv_r = wkv_d[:, :].rearrange("(k p) c -> p k c", p=128)
    bpl_r = bpl_d[:, :].rearrange("(t p) s -> p t s", p=128)

    with tile.TileContext(nc) as tc:
        with tc.tile_pool(name="wp", bufs=1) as wp, \
             tc.tile_pool(name="expp", bufs=6) as expp, \
             tc.tile_pool(name="caccp", bufs=3) as caccp, \
             tc.tile_pool(name="zrp", bufs=2) as zrp, \
             tc.tile_pool(name="stp", bufs=2, space="PSUM") as stp, \
             tc.tile_pool(name="avp", bufs=1, space="PSUM") as avp:

            # ---------------- persistent SBUF + input DMAs ----------------
            cst = wp.tile([128, 80], F32, tag="cst")
            xT = wp.tile([128, 4, L], BF16, tag="xT")
            wkv = wp.tile([128, 4, CH], BF16, tag="wkv")
            wq = wp.tile([128, 4, DIM], BF16, tag="wq")
            wo = wp.tile([128, 4, DIM], F32R, tag="wo")
            xTc = wp.tile([128, 4, 256], BF16, tag="xTc")
            bpl = wp.tile([128, 8, NS], BF16, tag="bpl")

            # DMA issue order == arrival order (single transfer channel).
            # Host permutes wkv cols / bpl tiles into consumption-pair order
            # [k0|v0|k1|v1|...] so each group's weights land as one
            # descriptor (HWDGE descriptor time dominates small DMAs).
            def load_wkv_grp(t):
                nc.sync.dma_start(out=wkv[:, :, t * 256:(t + 1) * 256],
                                  in_=wkv_r[:, :, t * 256:(t + 1) * 256])

            # first inputs dual-queued (SP + ACT hwdge) for descriptor
            # parallelism at startup
            nc.scalar.dma_start(out=wkv[:, :, 0:256], in_=wkv_r[:, :, 0:256])
            nc.scalar.dma_start(out=wq,
                                in_=wq_d[:, :].rearrange("(k p) c -> p k c", p=128))
            nc.sync.dma_start(out=cst, in_=cst_d[:, :])
            nc.sync.dma_start(out=xT[:, :, 0:512], in_=xT_r[:, :, 0:512])
            nc.sync.dma_start(out=xTc,
                              in_=xTc_d[:, :].rearrange("(k p) t -> p k t", p=128))
            nc.sync.dma_start(out=bpl[:, 0:2, :], in_=bpl_r[:, 0:2, :])
            nc.sync.dma_start(out=xT[:, :, 512:L], in_=xT_r[:, :, 512:L])
            load_wkv_grp(1)
            nc.sync.dma_start(out=bpl[:, 2:4, :], in_=bpl_r[:, 2:4, :])
            load_wkv_grp(2)
            nc.sync.dma_start(out=bpl[:, 4:8, :], in_=bpl_r[:, 4:8, :])
            load_wkv_grp(3)
            nc.sync.dma_start(out=wo,
                              in_=wo_d[:, :].rearrange("(k p) c -> p k c", p=128))

            # ---------------- setup (runs during DMA window) ----------------
            ident = wp.tile([128, 128], F32, tag="ident")
            make_identity(nc, ident)
            warm = wp.tile([1, 1], F32, tag="warm")
            nc.vector.memset(warm, 0.0)
            nc.scalar.activation(warm[:, :], warm[:, :], AF.Exp)

            # conv input planes: two pinned buffers, zero pad written once
            # (right pad col 32, bottom pad row 32; interior rewritten per tile)
            kvsb = [wp.tile([128, NPAD], BF16, tag=f"kvsb{i}", name=f"kvsb{i}")
                    for i in range(2)]
            zpad = wp.tile([128, PADW], F32, tag="zpad")
            nc.gpsimd.memset(zpad, 0.0)
            for i in range(2):
                pad_col = bass.AP(tensor=kvsb[i].tensor,
                                  offset=kvsb[i].offset + 32,
                                  ap=[kvsb[i].ap[0], [PADW, PADW]])
                nc.gpsimd.tensor_copy(pad_col, zpad[:, :])
                nc.gpsimd.tensor_copy(kvsb[i][:, PADW * 32: PADW * 32 + 32],
                                      zpad[:, 0:32])

            # V tiles [128 m, 128]: cols 0:64 = V^T chunk, cols 64:128 = ones
            # (AV matmul then yields softmax denominator on out partitions
            # 64:127 for free).
            vaug = wp.tile([128, 16, 128], BF16, tag="vaug")
            ones_f = wp.tile([128, 1024], F32, tag="ones_f")
            nc.gpsimd.memset(ones_f, 1.0)
            ones_dst = bass.AP(tensor=vaug.tensor, offset=vaug.offset + 64,
                               ap=[vaug.ap[0], [128, 16], [1, 64]])
            nc.gpsimd.tensor_copy(ones_dst,
                                  ones_f[:, :].rearrange("p (a b) -> p a b", b=64))

            kT2 = wp.tile([128, M], F32R, tag="kT2")
            qT2 = wp.tile([128, NH, 256], F32R, tag="qT2")
            attnT = [wp.tile([128, 256], F32R, tag=f"attnT{i}", name=f"attnT{i}")
                     for i in range(4)]
            ysb = wp.tile([128, 4, 256], F32, tag="ysb")

            # conv diagonal weights (Pool engine, from cst).  Emitted lazily,
            # one group ahead of use, so Pool's in-order queue stays
            # responsive for the kT2 copies the attention steps wait on.
            diags = {}

            def emit_diags(t):
                for t_i in (t, 4 + t):
                    for tap in range(6):
                        d = wp.tile([128, 128], BF16, tag=f"dg{t_i}_{tap}",
                                    name=f"dg{t_i}_{tap}")
                        nc.gpsimd.tensor_scalar_mul(
                            d[:, :], ident[:, :],
                            cst[:, 4 + 9 * t_i + tap: 5 + 9 * t_i + tap])
                        diags[(t_i, tap)] = d

            emit_diags(0)
            emit_diags(1)

            with tc.tile_pool(name="kvps", bufs=1, space="PSUM") as kvps, \
                 tc.tile_pool(name="cvps", bufs=1, space="PSUM") as cvps:

                # ---------------- pipeline building blocks ----------------
                def kv_chunk(t_i, buf, n, pad_dve):
                    """Project one 512-token chunk of ch-tile t_i into the
                    padded conv plane (rows 16n..16n+16).  The pad copy
                    alternates ACT/DVE to balance engine load."""
                    kvp = kvps.tile([128, 512], F32, tag="kvp")
                    wc = (2 * t_i if t_i < 4 else 2 * (t_i - 4) + 1) * 128
                    for k in range(4):
                        nc.tensor.matmul(kvp[:, :],
                                         wkv[:, k, wc:wc + 128],
                                         xT[:, k, n * 512:(n + 1) * 512],
                                         start=(k == 0), stop=(k == 3))
                    dst = bass.AP(tensor=buf.tensor,
                                  offset=buf.offset + PADW * 16 * n,
                                  ap=[buf.ap[0], [PADW, 16], [1, 32]])
                    src = kvp[:, :].rearrange("p (a b) -> p a b", b=32)
                    if pad_dve:
                        nc.vector.tensor_copy(dst, src)
                    else:
                        nc.scalar.copy(dst, src)

                def conv(t_i, buf, out_dtype, fuse=False):
                    # taps 0-5 on PE (diag matmuls); taps 6-8 as a DVE
                    # stt chain on top (saves 2.6us of PE in the PE-bound
                    # part A; DVE has slack).
                    cvp = cvps.tile([128, NS], F32, tag="cvp")
                    for tap in range(6):
                        dy, dx = tap // 3, tap % 3
                        win = bass.AP(tensor=buf.tensor,
                                      offset=buf.offset + PADW * dy + dx,
                                      ap=[buf.ap[0], [2 * PADW, KH], [2, KW]])
                        nc.tensor.matmul(cvp[:, :], diags[(t_i, tap)][:, :], win,
                                         start=(tap == 0), stop=(tap == 5))
                    # DVE accumulates taps 6-8 (seeded with the bias plane)
                    # in parallel with the PE taps; one combine at the end.
                    cacc = caccp.tile([128, NS], out_dtype, tag=f"cacc{out_dtype}")
                    bi = 2 * t_i if t_i < 4 else 2 * (t_i - 4) + 1
                    for tap in range(6, 9):
                        dy, dx = tap // 3, tap % 3
                        win = bass.AP(tensor=buf.tensor,
                                      offset=buf.offset + PADW * dy + dx,
                                      ap=[buf.ap[0], [2 * PADW, KH], [2, KW]])
                        nc.vector.scalar_tensor_tensor(
                            cacc[:, :], win,
                            cst[:, 4 + 9 * t_i + tap: 5 + 9 * t_i + tap],
                            bpl[:, bi, :] if tap == 6 else cacc[:, :],
                            op0=ALU.mult, op1=ALU.add)
                    if fuse:
                        return cvp, cacc
                    nc.vector.scalar_tensor_tensor(
                        cacc[:, :], cvp[:, :], 1.0, cacc[:, :],
                        op0=ALU.mult, op1=ALU.add)
                    return cacc

                def k_assemble(t, cvp, cacc):
                    # fused: kT2 = cvp (PE taps) + cacc (DVE taps + bias)
                    for gi in range(2):
                        mt0 = 4 * t + 2 * gi
                        nc.vector.scalar_tensor_tensor(
                            kT2[0:64, mt0 * 128:(mt0 + 2) * 128],
                            cvp[gi * 64:gi * 64 + 64, :], 1.0,
                            cacc[gi * 64:gi * 64 + 64, :],
                            op0=ALU.mult, op1=ALU.add)

                def v_assemble(t, cacc):
                    # PE transpose into an st-ring PSUM tile (no extra bank),
                    # then DVE copy (f32 -> bf16) into the vaug layout.
                    for gi in range(2):
                        vt = stp.tile([128, 1024], F32, tag="st")
                        for sh in range(2):
                            nc.tensor.transpose(
                                vt[:, sh * 512:sh * 512 + 64],
                                cacc[gi * 64:gi * 64 + 64,
                                     sh * 128:(sh + 1) * 128],
                                ident[gi * 64:gi * 64 + 64,
                                      gi * 64:gi * 64 + 64])
                        for sh in range(2):
                            mt = 4 * t + 2 * gi + sh
                            nc.vector.tensor_copy(vaug[:, mt, 0:64],
                                                  vt[:, sh * 512:sh * 512 + 64])

                def q_half(half):
                    qp = stp.tile([128, 1024], F32, tag="st")
                    for ti in range(2):
                        t_i = 2 * half + ti
                        for k in range(4):
                            nc.tensor.matmul(
                                qp[:, ti * 256:(ti + 1) * 256],
                                wq[:, k, t_i * 128:(t_i + 1) * 128],
                                xTc[:, k, :],
                                start=(k == 0), stop=(k == 3))
                    for ti in range(2):
                        t_i = 2 * half + ti
                        for gi in range(2):          # head 2*t_i + gi
                            h = 2 * t_i + gi
                            nc.vector.tensor_scalar_add(
                                qT2[0:64, h, :],
                                qp[gi * 64:gi * 64 + 64,
                                   ti * 256:(ti + 1) * 256],
                                cst[gi * 64:gi * 64 + 64, t_i:t_i + 1])

                def scores_step(mt, half):
                    """Scores + exp for one m-tile / head-half; returns ex."""
                    st = stp.tile([128, 1024], F32, tag="st")
                    for n in range(2):
                        nc.tensor.matmul(
                            st[:, n * 512:(n + 1) * 512],
                            kT2[0:64, mt * 128:(mt + 1) * 128],
                            qT2[0:64, 4 * half + 2 * n:4 * half + 2 * n + 2, :],
                            start=True, stop=True)
                    ex = expp.tile([128, 1024], BF16, tag="ex")
                    nc.scalar.activation(ex[:, :], st[:, :], AF.Exp,
                                         scale=float(SCALE))
                    return ex

                def av_step(mt, avt, ex):
                    for n in range(2):
                        nc.tensor.matmul(
                            avt[:, n * 512:(n + 1) * 512],
                            vaug[:, mt, :],
                            ex[:, n * 512:(n + 1) * 512],
                            start=(mt == 0), stop=(mt == 15))

                def normalize(half, avt):
                    """avt -> attnT[2*half], attnT[2*half+1] (cols = tokens).
                    Reciprocal split in 256-col chunks to shorten the DVE
                    serial chain at the tail."""
                    zr = zrp.tile([128, 1024], F32, tag="zr")
                    nc.vector.reciprocal(zr[0:64, :], avt[64:128, :])
                    for hh in range(4):              # head 4*half + hh
                        h = 4 * half + hh
                        nc.vector.scalar_tensor_tensor(
                            attnT[h // 2][(h % 2) * 64:(h % 2) * 64 + 64, :],
                            avt[0:64, hh * 256:(hh + 1) * 256], 1.0,
                            zr[0:64, hh * 256:(hh + 1) * 256],
                            op0=ALU.mult, op1=ALU.mult)

                # ---------------- part A: kv/conv/q + head-half 0 ----------------
                # Fine-grained weave: the exp latency of an attention step and
                # the pad-copy latency of a kv chunk are covered by emitting
                # the next block of independent PE work between the dependent
                # pairs (PE executes in emission order).
                avt0 = avp.tile([128, 1024], F32, tag="avt")

                def group(t):
                    f = [4 * (t - 1) + i for i in range(4)]   # fill steps
                    kv_chunk(t, kvsb[0], 0, pad_dve=False)
                    kv_chunk(t, kvsb[0], 1, pad_dve=False)
                    ex0 = scores_step(f[0], 0) if t > 0 else None
                    if t == 0:
                        q_half(0)
                    cvp_k, cacc_k = conv(t, kvsb[0], F32R, fuse=True)
                    if t > 0:
                        av_step(f[0], avt0, ex0)
                        ex1 = scores_step(f[1], 0)
                    k_assemble(t, cvp_k, cacc_k)
                    kv_chunk(4 + t, kvsb[1], 0, pad_dve=False)
                    kv_chunk(4 + t, kvsb[1], 1, pad_dve=False)
                    if t > 0:
                        av_step(f[1], avt0, ex1)
                        ex2 = scores_step(f[2], 0)
                    else:
                        q_half(1)
                    cacc_v = conv(4 + t, kvsb[1], F32)
                    if t > 0:
                        av_step(f[2], avt0, ex2)
                        ex3 = scores_step(f[3], 0)
                    v_assemble(t, cacc_v)
                    if t > 0:
                        av_step(f[3], avt0, ex3)
                    if t < 2:
                        emit_diags(t + 2)

                for t in range(4):
                    group(t)
                # drain the last m-group's half-0 steps (software pipelined)
                exq = scores_step(12, 0)
                for mt in range(13, 16):
                    exn = scores_step(mt, 0)
                    av_step(mt - 1, avt0, exq)
                    exq = exn
                av_step(15, avt0, exq)
                normalize(0, avt0)

            # ---------------- part B: head-half 1 + output proj ----------------
            # yp tiles pack two m-tiles per PSUM bank so all four output
            # accumulators are live; the wo-contraction partials for k=0,1
            # (attnT[0]/[1], ready since half 0) fill PE idle during the
            # ACT-paced half-1 attention steps.
            with tc.tile_pool(name="yps", bufs=1, space="PSUM") as yps:
                avt1 = avp.tile([128, 1024], F32, tag="avt")

                def yproj_pair(m, k0):
                    """wo-contraction partial (k0, k0+1) for m-tile m; a
                    closed PSUM group, accumulated into ysb by DVE.  Two
                    alternating tags so DVE accumulation never blocks the
                    next pair's matmuls."""
                    ypt = yps.tile([128, 256], F32, tag=f"yp{m % 2}",
                                   name=f"yp_{m}_{k0}")
                    for k in (k0, k0 + 1):
                        nc.tensor.matmul(ypt[:, :],
                                         wo[:, k, m * 128:(m + 1) * 128],
                                         attnT[k][:, :],
                                         start=(k == k0), stop=(k == k0 + 1))
                    if k0 == 0:
                        nc.vector.tensor_scalar_add(ysb[:, m, :], ypt[:, :],
                                                    cst[:, 76 + m:77 + m])
                    else:
                        nc.vector.scalar_tensor_tensor(
                            ysb[:, m, :], ypt[:, :], 1.0, ysb[:, m, :],
                            op0=ALU.mult, op1=ALU.add)

                exq = scores_step(0, 1)
                for mt in range(1, 16):
                    exn = scores_step(mt, 1)
                    av_step(mt - 1, avt1, exq)
                    exq = exn
                    if 4 <= mt < 8:
                        yproj_pair(mt - 4, 0)   # k=0,1 from half-0 heads
                av_step(15, avt1, exq)
                normalize(1, avt1)
                for m in range(4):
                    yproj_pair(m, 2)            # k=2,3 from half-1 heads
                    eng = nc.sync if m % 2 == 0 else nc.scalar
                    eng.dma_start(
                        out=y_d[m * 128:(m + 1) * 128, :],
                        in_=ysb[:, m, :])

    nc.finalize()
    return nc


def _get_program():
    if "nc" not in _NC_CACHE:
        _NC_CACHE["nc"] = _build_program()
    return _NC_CACHE["nc"]


def _host_prep(x, wq, bq, wkv, bkv, dw_kernel, dw_bias, wo, bo):
    """Build the 8 per-core input maps."""
    import ml_dtypes
    BF = ml_dtypes.bfloat16
    x = np.ascontiguousarray(np.asarray(x, np.float32))
    wq_r = np.asarray(wq, np.float32).astype(BF)
    wkv_p = np.asarray(wkv, np.float32).reshape(DIM, 8, 128)
    perm = [0, 4, 1, 5, 2, 6, 3, 7]          # [k0|v0|k1|v1|...]
    wkv_r = np.ascontiguousarray(wkv_p[:, perm, :].reshape(DIM, CH)).astype(BF)
    wo_r = _round_f32r(np.asarray(wo, np.float32))
    bq = np.asarray(bq, np.float32)
    bkv = np.asarray(bkv, np.float32)
    dw_bias = np.asarray(dw_bias, np.float32)
    bo = np.asarray(bo, np.float32)
    dww = np.asarray(dw_kernel, np.float32).reshape(9, CH).T.copy()  # [1024, 9]

    # bias plane: dw_bias + bkv * sum(valid taps), SAME padding aware
    oy = np.arange(KH)
    valid_y = (2 * oy[:, None] + np.arange(3)[None, :]) < H      # [16, 3]
    valid_x = valid_y.copy()
    wsum = np.zeros((CH, KH, KW), np.float32)
    for tap in range(9):
        dy, dx = tap // 3, tap % 3
        m2 = np.outer(valid_y[:, dy], valid_x[:, dx]).astype(np.float32)
        wsum += dww[:, tap][:, None, None] * m2[None, :, :]
    bpl = (dw_bias[:, None] + bkv[:, None] * wsum.reshape(CH, NS))
    bpl = np.ascontiguousarray(
        bpl.reshape(8, 128, NS)[[0, 4, 1, 5, 2, 6, 3, 7]].reshape(CH, NS)).astype(BF)

    cst = np.zeros((128, 80), np.float32)
    cst[:, 0:4] = bq.reshape(4, 128).T
    for t_i in range(8):
        cst[:, 4 + 9 * t_i: 13 + 9 * t_i] = dww[t_i * 128:(t_i + 1) * 128, :]
    cst[:, 76:80] = bo.reshape(4, 128).T

    in_maps = []
    for c in range(8):
        b, j = c // 4, c % 4
        xT = x[b].reshape(L, DIM).T.astype(BF)
        in_maps.append({
            "xT": np.ascontiguousarray(xT),
            "xTc": np.ascontiguousarray(xT[:, j * 256:(j + 1) * 256]),
            "wkv": wkv_r, "wq": wq_r, "wo": wo_r,
            "bpl": bpl, "cst": cst,
        })
    return in_maps


def kernel(**inputs) -> np.ndarray:
    nc = _get_program()
    in_maps = _host_prep(**inputs)
    res = run_bass_kernel_spmd(nc, in_maps, core_ids=list(range(8)))
    out = np.zeros((B, H, W, DIM), np.float32)
    flat = out.reshape(B, L, DIM)
    for c in range(8):
        b, j = c // 4, c % 4
        flat[b, j * 256:(j + 1) * 256, :] = res.results[c]["y"].T
    return out
